# revision 7
# baseline (speedup 1.0000x reference)
"""Trainium2 Bass kernel for speaker-rate positional-encoding attention (v2).

Math (per batch b):
  rate_q = sigmoid(spk @ Wsq.T + bsq);  rate_k = sigmoid(spk @ Wsk.T + bsk)
  pe(x)[l,d] = sin(rate * pos[l] * 10000^(-d/D) + phase[d]),  phase = 0/pi/2
  x  = query + pe_q                      (NO projection needed -- folded)
  k2 = (keys + pe_k) @ (Wk.T @ Wq) + bk @ Wq        [weight folding]
  scores[t,s] = x[t] . k2[s] (+ c[s] if bq != 0)  == q[t] . k[s]
  vpp = values @ (Wo @ Wv).T  (+ Wo @ bv)           [weight folding]
  E = exp(scores^T / sqrt(D));  out[t] = (sum_s E[s,t] vpp[s,:]) /
      (sqrt(D) * sum_s E[s,t]) + (bo + Wo@bv/sqrt(D))

v2 changes over v1 (177.7us measured):
 - All-bf16 datapath (q/xT/k2T bf16): rel err ~1.1e-2 < 2e-2 budget;
   halves input DMA and SBUF, enables FWL everywhere.
 - Posenc via angle-addition anchors: pe[l] = sinA[l//32]*cosB[l%32]
   + cosA[l//32]*sinB[l%32] (phase folded into B).  ACT computes sin
   on 32+32 anchors per (batch,side) instead of the full [128,L] grid:
   ~40us of ACT work and 3 table swaps removed.  The [128,L] outer
   products run on DVE via stride-0 broadcast access patterns, in bf16.
 - sigmoid via tanh (0.5+0.5*tanh(z/2)): tanh+sin share one ACT table
   set (silu_and_others), exp loads once -> 2 table loads total.
 - softmax row-sums fold into the out matmul: vpp carries a 257th
   column of sqrt(D) per 256-wide e-chunk, so psF[:,256] = sqrtD*SumE
   and recip comes from PSUM directly (v1 spent ~11us of PE time on
   64 tiny ones-matmuls per core).
 - DMA: all tensors repacked so every transfer is a straight
   [P, contiguous] copy with 4-16KB descriptors; weights split across
   the scalar ring (wkq) and sync ring (wvo) issued at t=0; inputs on
   gpsimd (k/v) and vector (q) rings; out written bf16 (halved).
"""

import sys

for _p in ("/opt/trn_rl_repo",):
    if _p not in sys.path:
        sys.path.insert(0, _p)

import numpy as np

import concourse.bass as bass
from concourse import bacc
import concourse.mybir as mybir
import concourse.tile as tile
from concourse.bass_utils import run_bass_kernel_spmd

B, T, S, D, SPK = 16, 1024, 512, 1024, 256
NCORES = 8
BL = B // NCORES          # batches per core
P = 128
DT = D // P               # 8 d-tiles (contraction chunks / m-tiles)
ST = S // P               # 4 s-tiles
TT = T // P               # 8 t-tiles
C = 32                    # posenc anchor block (l = C*i + j)
NIQ = T // C              # 32 A-anchors (query side)
NIK = S // C              # 16 A-anchors (key side)
F32 = mybir.dt.float32
BF16 = mybir.dt.bfloat16
I32 = mybir.dt.int32
SQRT_D = float(np.sqrt(D))
PI = float(np.pi)
TWO_PI = 2.0 * PI

AF = mybir.ActivationFunctionType
ALU = mybir.AluOpType

WARM_SPINS = 40

# Sin scale: marginally under 2*pi so |scale*res| stays inside the ACT
# sin table's documented [-pi, pi] domain for res in [-0.5, 0.5].
SIN_SCALE = 6.283185

# cblk column layout (f32, [P, CBLK_N])
CB_INVD2P = 0          # 8: invdiv/2pi per dt
CB_INVD2PH = 8         # 8: invdiv/4pi (for 0.5*(1+tanh) fold)
CB_QUARTER = 16        # 1: 0.25 (cos via sin(2pi*(x+1/4)))
CB_PHS2P = 17          # 1: phi/2pi, phi_p = (p%2)*pi/2
CB_PHS2P25 = 18        # 1: phi/2pi + 0.25
CB_BKQ = 20            # 8: (bk @ Wq) per (mt) column
CB_SPK = 28            # 4: speaker embedding packed [kt*BL + b]
CB_WSQ = 32            # 256
CB_WSK = 288           # 256
CB_BSQH = 544          # 1: bsq/2
CB_BSKH = 545          # 1: bsk/2
CB_PAQ = 546           # 32: mel0 + 32*i
CB_PAK = 578           # 16: 32*i
CB_PB = 594            # 32: j
CBLK_N = 626


def ap0(sl, n, where):
    """Append a stride-0 (broadcast) free dim of size n to an AP.

    'inner': new innermost dim (varies fastest); 'after': right after the
    partition dim (varies slowest among free dims)."""
    if where == "inner":
        return bass.AP(tensor=sl.tensor, offset=sl.offset, ap=[*sl.ap, [0, n]])
    return bass.AP(tensor=sl.tensor, offset=sl.offset,
                   ap=[sl.ap[0], [0, n], *sl.ap[1:]])


def bcast(ap, n=P):
    # replicate a DRAM vector across n partitions (DMA partition-step 0)
    return bass.AP(tensor=ap.tensor, offset=ap.offset, ap=[[0, n], *ap.ap])


def build_nc(use_c=False, cconst=0.0):
    nc = bacc.Bacc()
    dp = nc.declare_dram_parameter
    qTi = dp("qTi", [BL, P, DT * T], BF16, isOutput=False)  # q^T  [b,p_d,dt*T+t]
    kTi = dp("kTi", [BL, P, DT * S], BF16, isOutput=False)
    vTi = dp("vTi", [BL, P, DT * S], BF16, isOutput=False)
    wkq = dp("wkq", [P, DT * D], BF16, isOutput=False)      # (Wk.T@Wq)[d, m]
    wvo = dp("wvo", [P, DT * D], BF16, isOutput=False)      # (Wo@Wv).T[d, e]
    biasf = dp("biasf", [D], F32, isOutput=False)           # sqrt(D)*bo + Wo@bv
    cblk = dp("cblk", [P, CBLK_N], F32, isOutput=False)
    ucd = dp("ucd", [P, DT], BF16, isOutput=False)          # bq @ Wk (c-term)
    out = dp("out", [BL, TT, P, D], BF16, isOutput=True)

    with tile.TileContext(nc) as tc:
        with (
            tc.tile_pool(name="consts", bufs=1) as cp,
            tc.tile_pool(name="tmp", bufs=3) as tp,
            tc.tile_pool(name="big", bufs=2) as bp,
            tc.tile_pool(name="outp", bufs=3) as op,
            tc.tile_pool(name="psum", bufs=7 if use_c else 8, space="PSUM") as pp,
        ):
            # ---------------- front DMAs (issue everything at t=0) -------
            cblk_sb = cp.tile([P, CBLK_N], F32)
            nc.sync.dma_start(out=cblk_sb, in_=cblk[:])
            biasf_bc = cp.tile([P, D], F32)
            nc.sync.dma_start(out=biasf_bc, in_=bcast(biasf[:]))
            wvo_sb = cp.tile([P, DT, D], BF16)
            for h in range(2):
                nc.sync.dma_start(out=wvo_sb[:, 4 * h:4 * h + 4],
                                  in_=wvo[:, 4 * h * D:(4 * h + 4) * D])
            if use_c:
                uc_sb = cp.tile([P, DT], BF16)
                nc.sync.dma_start(out=uc_sb, in_=ucd[:])

            # scalar ring: wkq first (k2 b0 streams it), then q for batch 1
            wkq_sb = cp.tile([P, DT, D], BF16)
            for h in range(2):
                nc.scalar.dma_start(out=wkq_sb[:, 4 * h:4 * h + 4],
                                    in_=wkq[:, 4 * h * D:(4 * h + 4) * D])
            xT = [bp.tile([P, DT, T], BF16, tag="xT", name=f"xT{b}")
                  for b in range(BL)]
            for h in range(2):
                nc.scalar.dma_start(
                    out=xT[1][:, 4 * h:4 * h + 4],
                    in_=qTi[1][:, 4 * h * T:(4 * h + 4) * T])

            # gpsimd ring: batch-0 inputs (v, k, q), then batch-1 v/k
            vT = [bp.tile([P, DT, S], BF16, tag="vT", name=f"vT{b}")
                  for b in range(BL)]
            xkT = [bp.tile([P, DT, S], BF16, tag="xkT", name=f"xkT{b}")
                   for b in range(BL)]
            for h in range(2):
                nc.gpsimd.dma_start(
                    out=vT[0][:, 4 * h:4 * h + 4],
                    in_=vTi[0][:, 4 * h * S:(4 * h + 4) * S])
                nc.gpsimd.dma_start(
                    out=xkT[0][:, 4 * h:4 * h + 4],
                    in_=kTi[0][:, 4 * h * S:(4 * h + 4) * S])
            for h in range(2):
                nc.gpsimd.dma_start(
                    out=xT[0][:, 4 * h:4 * h + 4],
                    in_=qTi[0][:, 4 * h * T:(4 * h + 4) * T])
            for h in range(2):
                nc.gpsimd.dma_start(
                    out=vT[1][:, 4 * h:4 * h + 4],
                    in_=vTi[1][:, 4 * h * S:(4 * h + 4) * S])
                nc.gpsimd.dma_start(
                    out=xkT[1][:, 4 * h:4 * h + 4],
                    in_=kTi[1][:, 4 * h * S:(4 * h + 4) * S])

            # ---------------- small consts ----------------
            invd2p = cblk_sb[:, CB_INVD2P:CB_INVD2P + 8]
            invd2ph = cblk_sb[:, CB_INVD2PH:CB_INVD2PH + 8]
            quarter = cblk_sb[:, CB_QUARTER:CB_QUARTER + 1]
            phs2p = cblk_sb[:, CB_PHS2P:CB_PHS2P + 1]
            phs2p25 = cblk_sb[:, CB_PHS2P25:CB_PHS2P25 + 1]
            bkq_sb = cblk_sb[:, CB_BKQ:CB_BKQ + 8]
            spk_sb = cblk_sb[:, CB_SPK:CB_SPK + 2 * BL]
            wsq_sb = cblk_sb[:, CB_WSQ:CB_WSQ + 256]
            wsk_sb = cblk_sb[:, CB_WSK:CB_WSK + 256]
            bsqh = cblk_sb[:, CB_BSQH:CB_BSQH + 1]
            bskh = cblk_sb[:, CB_BSKH:CB_BSKH + 1]
            pAq = cblk_sb[:, CB_PAQ:CB_PAQ + NIQ]
            pAk = cblk_sb[:, CB_PAK:CB_PAK + NIK]
            pB = cblk_sb[:, CB_PB:CB_PB + C]

            zero_b = cp.tile([P, 1], F32)
            nc.vector.memset(zero_b, 0.0)

            # ---------------- PE warmup spin ----------------
            # HAM clock-gate: keep the PE array busy through the DMA front
            # so it is at 2.4GHz when real matmuls start.
            warm_a = cp.tile([P, P], BF16)
            nc.vector.memset(warm_a, 0.0)
            warm_b = cp.tile([P, 512], BF16)
            nc.vector.memset(warm_b, 0.0)
            for w in range(8):
                psW = pp.tile([P, 512], F32, tag="ps", name=f"psW{w}")
                nc.tensor.matmul(psW, warm_a, warm_b, start=True, stop=True)

            # ---------------- speaker rates (tanh == sigmoid fold) -------
            # psum[p, b] = sum_d Ws[d] * spk[b, d]   (replicated over p)
            # rate = sigmoid(z + bs) = 0.5*(1 + tanh(z/2 + bs/2)); the 0.5s
            # fold into srX2 below.  tanh and sin share one ACT table set.
            th_sb = cp.tile([P, 2 * BL], F32)
            for which, wmat, biash in ((0, wsq_sb, bsqh), (1, wsk_sb, bskh)):
                ps = pp.tile([P, 512], F32, tag="ps")
                for kt in range(2):
                    nc.tensor.matmul(
                        ps[:, :BL],
                        wmat[:, kt * P:(kt + 1) * P],
                        spk_sb[:, kt * BL:(kt + 1) * BL],
                        start=(kt == 0),
                        stop=(kt == 1),
                    )
                nc.scalar.activation(
                    th_sb[:, which * BL:(which + 1) * BL], ps[:, :BL],
                    AF.Tanh, bias=biash, scale=0.5,
                )
            for w in range(8, WARM_SPINS):
                psW = pp.tile([P, 512], F32, tag="ps", name=f"psW{w}")
                nc.tensor.matmul(psW, warm_a, warm_b, start=True, stop=True)

            # srX2[p, b, dt] = rate * invdiv / 2pi = invd2ph*(1 + tanh)
            srq2 = cp.tile([P, BL, DT], F32)
            srk2 = cp.tile([P, BL, DT], F32)
            for b in range(BL):
                nc.vector.scalar_tensor_tensor(
                    srq2[:, b], invd2ph, th_sb[:, b:b + 1], invd2ph,
                    ALU.mult, ALU.add)
                nc.vector.scalar_tensor_tensor(
                    srk2[:, b], invd2ph, th_sb[:, BL + b:BL + b + 1], invd2ph,
                    ALU.mult, ALU.add)

            # ---------------- posenc anchors (both batches up front) -----
            # theta[l,d] = r*w_d*(p0+l) + phi = A_i + (B_j + phi), l = 32i+j
            # For each trig output the shift (0 / 1/4 for cos / phi/2pi)
            # folds into the angle BEFORE the f32->i32 rint, so the final
            # sin argument 2pi*(x+s-rint(x+s)) always lies in [-pi, pi].
            anch = {}
            for b in range(BL):
                for side, sr2, pA, ni in (("q", srq2, pAq, NIQ),
                                          ("k", srk2, pAk, NIK)):
                    sA = cp.tile([P, DT, ni], BF16, name=f"sA{side}{b}")
                    cA = cp.tile([P, DT, ni], BF16, name=f"cA{side}{b}")
                    sB = cp.tile([P, DT, C], BF16, name=f"sB{side}{b}")
                    cB = cp.tile([P, DT, C], BF16, name=f"cB{side}{b}")
                    anch[(b, side)] = (sA, cA, sB, cB)
                    for pvec, n, ss, cc, sshift, cshift in (
                        (pA, ni, sA, cA, zero_b, quarter),
                        (pB, C, sB, cB, phs2p, phs2p25),
                    ):
                        ang = tp.tile([P, DT * n], F32, tag="ang",
                                      name=f"ang{side}{b}{n}")
                        # ang[p, dt, i] = sr2[p,dt] * pvec[p,i]
                        nc.vector.tensor_tensor(
                            out=ang,
                            in0=ap0(sr2[:, b], n, "inner"),
                            in1=ap0(pvec, DT, "after"),
                            op=ALU.mult)
                        for trig, shift in ((ss, sshift), (cc, cshift)):
                            yi = tp.tile([P, DT * n], I32, tag="yi", bufs=2,
                                         name=f"yi{side}{b}{n}{trig is cc}")
                            nc.scalar.activation(yi, ang, AF.Identity,
                                                 bias=shift, scale=1.0)
                            res = tp.tile([P, DT * n], F32, tag="res", bufs=2,
                                          name=f"res{side}{b}{n}{trig is cc}")
                            nc.vector.scalar_tensor_tensor(
                                res, ang, shift, yi, ALU.add, ALU.subtract)
                            nc.scalar.activation(trig, res, AF.Sin,
                                                 bias=zero_b, scale=SIN_SCALE)

            # pe apply: dst[:, dt, :] += sinA (x) cosB' + cosA (x) sinB'
            # (outer products over (i, j) via stride-0 broadcast APs, bf16)
            def pe_add(b, side, dst, dt, ni):
                sA, cA, sB, cB = anch[(b, side)]
                L = ni * C
                for t1, t2 in ((sA, cB), (cA, sB)):
                    prod = tp.tile([P, L], BF16, tag=f"prod{side}", bufs=2,
                                   name=f"prod{side}{b}_{dt}_{t1 is sA}")
                    nc.vector.tensor_tensor(
                        out=prod,
                        in0=ap0(t1[:, dt], C, "inner"),
                        in1=ap0(t2[:, dt], ni, "after"),
                        op=ALU.mult)
                    nc.vector.tensor_add(dst[:, dt], dst[:, dt], prod)

            # ---------------- per-batch pipeline ----------------
            k2T = [bp.tile([P, DT, S], BF16, tag="k2T", name=f"k2T{b}")
                   for b in range(BL)]
            vpp = [bp.tile([P, ST, 4, 257], BF16, tag="vpp", name=f"vpp{b}")
                   for b in range(BL)]
            Et = [bp.tile([P, ST, T], BF16, tag="Et", name=f"Et{b}")
                  for b in range(BL)]
            recip = [bp.tile([P, TT], F32, tag="recip", name=f"recip{b}")
                     for b in range(BL)]

            def vpp_phase(b):
                # psV[i=(st,ech)] [128 s, 512 e] = sum_dt vT.T @ wvo
                psV = [pp.tile([P, 512], F32, tag="ps", name=f"psV{b}_{i}")
                       for i in range(2 * ST)]

                def vpp_mm(i, dt):
                    st, ech = divmod(i, 2)
                    nc.tensor.matmul(
                        psV[i],
                        vT[b][:, dt, st * P:(st + 1) * P],
                        wvo_sb[:, dt, ech * 512:(ech + 1) * 512],
                        start=(dt == 0),
                        stop=(dt == DT - 1),
                    )

                if b == 0:
                    for dt in range(DT):
                        for i in range(2 * ST):
                            vpp_mm(i, dt)
                else:
                    for i in range(2 * ST):
                        for dt in range(DT):
                            vpp_mm(i, dt)
                return psV

            def vpp_drains(b, psV):
                # ones columns (sqrt(D)) for the folded softmax row-sums
                vp = vpp[b]
                base = vp[:, 0, 0, 256:257]
                ones_ap = bass.AP(tensor=base.tensor, offset=base.offset,
                                  ap=[base.ap[0], [257, 4 * ST]])
                nc.vector.memset(ones_ap, SQRT_D)
                # vpp' = psV + sqrt(D)*biasf  (256-wide chunks)
                for i in range(2 * ST):
                    st, ech = divmod(i, 2)
                    for c2 in range(2):
                        sl = slice(256 * c2, 256 * c2 + 256)
                        esl = slice(512 * ech + 256 * c2,
                                    512 * ech + 256 * c2 + 256)
                        nc.vector.tensor_add(
                            vpp[b][:, st, 2 * ech + c2, 0:256],
                            psV[i][:, sl], biasf_bc[:, esl])

            def k2_phase(b):
                psK = [pp.tile([P, 512], F32, tag="ps", name=f"psK{b}_{i}")
                       for i in range(DT)]

                def k2_mm(mt, dt):
                    nc.tensor.matmul(
                        psK[mt][:, :S],
                        wkq_sb[:, dt, mt * P:(mt + 1) * P],
                        xkT[b][:, dt],
                        start=(dt == 0),
                        stop=(dt == DT - 1),
                    )

                if b == 0:
                    for dt in range(DT):
                        for mt in range(DT):
                            k2_mm(mt, dt)
                else:
                    for mt in range(DT):
                        for dt in range(DT):
                            k2_mm(mt, dt)
                psC = None
                if use_c:
                    psC = pp.tile([P, 512], F32, tag="psc", name=f"psC{b}",
                                  bufs=1)
                    for dt in range(DT):
                        for st in range(ST):
                            nc.tensor.matmul(
                                psC[:, st:st + 1],
                                xkT[b][:, dt, st * P:(st + 1) * P],
                                uc_sb[:, dt:dt + 1],
                                start=(dt == 0),
                                stop=(dt == DT - 1),
                                skip_group_check=True,
                            )
                return psK, psC

            def k2_drains(b, psK, psC):
                for mt in range(DT):
                    nc.scalar.activation(
                        k2T[b][:, mt], psK[mt][:, :S], AF.Identity,
                        bias=bkq_sb[:, mt:mt + 1], scale=1.0)
                if use_c:
                    cb_sb = cp.tile([P, ST], F32, name=f"cb{b}", tag=f"cb{b}")
                    nc.vector.tensor_scalar(
                        cb_sb, psC[:, :ST], cconst, 1.0 / SQRT_D,
                        ALU.add, ALU.mult)
                    return cb_sb
                return None

            def scores_phase(b, cb_sb):
                psS = [pp.tile([P, 512], F32, tag="ps", name=f"psS{b}_{i}")
                       for i in range(2 * ST)]
                for mt in range(DT):
                    for i in range(2 * ST):
                        tc2, st = divmod(i, ST)
                        nc.tensor.matmul(
                            psS[i],
                            k2T[b][:, mt, st * P:(st + 1) * P],
                            xT[b][:, mt, tc2 * 512:(tc2 + 1) * 512],
                            start=(mt == 0),
                            stop=(mt == DT - 1),
                        )
                for i in range(2 * ST):
                    tc2, st = divmod(i, ST)
                    sl = slice(tc2 * 512, (tc2 + 1) * 512)
                    nc.scalar.activation(
                        Et[b][:, st, sl], psS[i], AF.Exp,
                        bias=cb_sb[:, st:st + 1] if use_c else zero_b,
                        scale=1.0 / SQRT_D)

            def out_phase(b):
                # psF[t, 0:256] = sum_s Et*vpp-chunk; psF[t,256] = sqrtD*SumE
                for tt in range(TT):
                    osb = op.tile([P, D], BF16, tag="osb", name=f"osb{b}_{tt}")
                    psFs = []
                    for ec4 in range(4):
                        psF = pp.tile([P, 512], F32, tag="ps",
                                      name=f"psF{b}_{tt}_{ec4}")
                        psFs.append(psF)
                        for st in range(ST):
                            nc.tensor.matmul(
                                psF[:, 0:257],
                                Et[b][:, st, tt * P:(tt + 1) * P],
                                vpp[b][:, st, ec4],
                                start=(st == 0),
                                stop=(st == ST - 1),
                            )
                    nc.vector.reciprocal(recip[b][:, tt:tt + 1],
                                         psFs[0][:, 256:257])
                    for ec4 in range(4):
                        sl = slice(ec4 * 256, (ec4 + 1) * 256)
                        if ec4 < 2:
                            nc.vector.tensor_scalar_mul(
                                osb[:, sl], psFs[ec4][:, 0:256],
                                recip[b][:, tt:tt + 1])
                        else:
                            nc.scalar.activation(
                                osb[:, sl], psFs[ec4][:, 0:256], AF.Identity,
                                bias=zero_b, scale=recip[b][:, tt:tt + 1])
                    nc.sync.dma_start(out=out[b, tt], in_=osb)

            # ---- batch 0 ----
            for dt in range(DT):
                pe_add(0, "k", xkT[0], dt, NIK)
            psV0 = vpp_phase(0)
            psK0, psC0 = k2_phase(0)
            for dt in range(DT):
                pe_add(0, "q", xT[0], dt, NIQ)
            vpp_drains(0, psV0)
            cb0 = k2_drains(0, psK0, psC0)
            # early b1 key-side posenc on the DVE queue (inputs land ~30us)
            for dt in range(DT):
                pe_add(1, "k", xkT[1], dt, NIK)
            scores_phase(0, cb0)
            out_phase(0)
            # ---- batch 1 ----
            for dt in range(DT):
                pe_add(1, "q", xT[1], dt, NIQ)
            psV1 = vpp_phase(1)
            psK1, psC1 = k2_phase(1)
            vpp_drains(1, psV1)
            cb1 = k2_drains(1, psK1, psC1)
            scores_phase(1, cb1)
            out_phase(1)
    return nc


def marshal_inputs(query, keys, values, speaker_embedding, Wsq, bsq, Wsk, bsk,
                   Wq, bq, Wk, bk, Wv, bv, Wo, bo, current_mel_pos):
    import ml_dtypes
    BF = ml_dtypes.bfloat16
    f = lambda x: np.ascontiguousarray(np.asarray(x, dtype=np.float32))
    query, keys, values = f(query), f(keys), f(values)
    spk = f(speaker_embedding)
    Wsq, Wsk = f(Wsq), f(Wsk)
    Wq, Wk, Wv, Wo = f(Wq), f(Wk), f(Wv), f(Wo)
    bq, bk, bv, bo = f(bq), f(bk), f(bv), f(bo)
    bsq, bsk = f(bsq), f(bsk)
    mel0 = int(np.asarray(current_mel_pos).item())

    dvec = np.arange(D, dtype=np.float32)
    invdiv = (10000.0 ** (-dvec / D)).astype(np.float32)
    phase = np.where(dvec.astype(np.int64) % 2 == 0, 0.0,
                     np.pi / 2).astype(np.float32)

    # weight folding (host: weights only, no activation math)
    Wkq = (Wk.T @ Wq).astype(np.float32)            # k2 = xk @ Wkq
    Wvo = (Wo @ Wv).astype(np.float32)              # vpp = values @ Wvo.T
    bkq_v = (bk @ Wq).astype(np.float32)
    biasf_v = (SQRT_D * bo + (Wo @ bv)).astype(np.float32)
    uc_v = (bq @ Wk).astype(np.float32)
    cconst = float(bq @ bk)
    use_c = bool(np.any(bq))

    col = lambda v: np.ascontiguousarray(v.reshape(DT, P).T)
    rep = lambda v: np.repeat(np.asarray(v, np.float32)[None, :], P, axis=0)
    wsq_rep = np.ascontiguousarray(
        np.repeat(Wsq.reshape(2, P, 1), P, axis=2)
        .transpose(1, 0, 2).reshape(P, 2 * P))
    wsk_rep = np.ascontiguousarray(
        np.repeat(Wsk.reshape(2, P, 1), P, axis=2)
        .transpose(1, 0, 2).reshape(P, 2 * P))

    pcol = np.arange(P, dtype=np.float32)
    phs_col = ((pcol.astype(np.int64) % 2) * (np.pi / 2)).astype(np.float32)

    cblk_base = np.zeros((P, CBLK_N), np.float32)
    cblk_base[:, CB_INVD2P:CB_INVD2P + 8] = col(invdiv / TWO_PI)
    cblk_base[:, CB_INVD2PH:CB_INVD2PH + 8] = col(invdiv / (2 * TWO_PI))
    cblk_base[:, CB_QUARTER] = 0.25
    cblk_base[:, CB_PHS2P] = phs_col / TWO_PI
    cblk_base[:, CB_PHS2P25] = phs_col / TWO_PI + 0.25
    cblk_base[:, CB_BKQ:CB_BKQ + 8] = col(bkq_v)
    cblk_base[:, CB_WSQ:CB_WSQ + 256] = wsq_rep
    cblk_base[:, CB_WSK:CB_WSK + 256] = wsk_rep
    cblk_base[:, CB_BSQH] = bsq.reshape(-1)[0] / 2
    cblk_base[:, CB_BSKH] = bsk.reshape(-1)[0] / 2
    cblk_base[:, CB_PAQ:CB_PAQ + NIQ] = rep(mel0 + C * np.arange(NIQ))
    cblk_base[:, CB_PAK:CB_PAK + NIK] = rep(C * np.arange(NIK))
    cblk_base[:, CB_PB:CB_PB + C] = rep(np.arange(C))

    packW = lambda w: np.ascontiguousarray(
        w.reshape(DT, P, D).transpose(1, 0, 2).reshape(P, DT * D)).astype(BF)
    shared = {
        "wkq": packW(Wkq),
        "wvo": packW(np.ascontiguousarray(Wvo.T)),
        "biasf": biasf_v,
        "ucd": col(uc_v).astype(BF),
    }
    # [BL_total, L, D] -> per-core [BL, P, DT*L] with x[b, p, dt*L + l]
    tr = lambda x, L: np.ascontiguousarray(
        x.reshape(-1, L, DT, P).transpose(0, 3, 2, 1).reshape(-1, P, DT * L))
    qT_all = tr(query, T).astype(BF)
    kT_all = tr(keys, S).astype(BF)
    vT_all = tr(values, S).astype(BF)
    in_maps = []
    for c in range(NCORES):
        sl = slice(c * BL, (c + 1) * BL)
        m = dict(shared)
        m["qTi"] = np.ascontiguousarray(qT_all[sl])
        m["kTi"] = np.ascontiguousarray(kT_all[sl])
        m["vTi"] = np.ascontiguousarray(vT_all[sl])
        cb = cblk_base.copy()
        cb[:, CB_SPK:CB_SPK + 2 * BL] = np.ascontiguousarray(
            spk[sl].T.reshape(2, P, BL).transpose(1, 0, 2).reshape(P, 2 * BL))
        m["cblk"] = cb
        in_maps.append(m)
    build_args = dict(use_c=use_c, cconst=cconst)
    return in_maps, build_args


def run_device(in_maps, build_args=None, trace=False, **kw):
    nc = build_nc(**(build_args or {}))
    if not nc.is_finalized():
        nc.finalize()
    res = run_bass_kernel_spmd(nc, in_maps, core_ids=list(range(NCORES)),
                               trace=trace, **kw)
    outs = [np.asarray(r["out"]).astype(np.float32).reshape(BL, T, D)
            for r in res.results]
    return np.concatenate(outs, axis=0), res


def kernel(**inputs) -> np.ndarray:
    in_maps, build_args = marshal_inputs(**inputs)
    out, _ = run_device(in_maps, build_args)
    return out
